# revision 38
# baseline (speedup 1.0000x reference)
"""Fused Linear + LayerNorm + residual-multiply kernel for 8 Trainium2 cores.

Computes, for full inputs x[B,1024], y[B,1024], weight[1024,1024], bias, gamma, beta:
    z  = x @ weight.T + bias
    ln = (z - mean(z)) * rsqrt(var(z) + eps) * gamma + beta     (over last dim)
    out = (ln + y) * y
Data-parallel over the batch dim: each of the 8 NeuronCores processes B/8 rows.

v2: fp8 DoubleRow matmuls. Host casts x*SX and W.T*SW to fp8e4 (e4m3fn) and
packs them in k-pair layout; the PE runs DoubleRow matmuls (2 k-subtiles per
instruction, 0.5 cyc/row) so the z = x@W.T contraction streams in 1/4 the fp16
cycles. The scale c = SX*SW rides through PSUM and is undone by the LayerNorm
itself (mean/std scale together); only eps must be pre-scaled by c^2. Bias is
added in PSUM via a K=1 fp8 DoubleRow matmul from a ones row.

Per-core per 128-row tile (P=128, D=1024):
  - PE: bias matmul (start=True) + 4 k-pair x 2 half DoubleRow matmuls.
  - ScalarE: Copy PSUM f32 -> SBUF fp16 z_sb (so the DVE tail runs in 16-bit
    2x mode); later the Sqrt(var*1 + c^2 eps) activation.
  - VectorE: bn_stats/bn_aggr on z_sb for mean/var, reciprocal, -mean*rstd,
    then one fused AFFINE_THEN_ADD: u = (z_sb*rstd + nmr) + y  (= ln + y).
  - GpSimd: o = u * y (fp16), freeing the DVE for the next tile.
  - DMA: x/y loads on the sync HWDGE ring; W/bias once + out stores on the
    scalar ring (out alternates rings to balance sequencer time).
Output is fp16 in DRAM; the host upcasts to fp32. Total HBM traffic/core:
2MB x + 4MB y + 1MB W + 4MB out = 11MB.
"""

import numpy as np
import ml_dtypes
from contextlib import ExitStack

import concourse.bass as bass
import concourse.mybir as mybir
import concourse.tile as tile
from concourse import bacc, bass_utils

P = 128
D = 1024
KP = 4               # k-pairs (each pair = 2 k-subtiles of 128 = 256 contraction)
OB = 512             # o-block width (one PSUM bank of fp32)
ST = 512             # rows per x.T super-chunk
N_CORES = 8
EPS = 1e-5
SX = 4.0             # fp8 scale on x
SW = 64.0            # fp8 scale on W
SC = SX * SW         # PSUM carries SC * z

F32 = mybir.dt.float32
F16 = mybir.dt.float16
F8 = mybir.dt.float8e4

AF = mybir.ActivationFunctionType
OP = mybir.AluOpType
DR = mybir.MatmulPerfMode.DoubleRow

_BUILD_CACHE = {}


def _build(b_core: int, trivial_affine: bool):
    key = (b_core, trivial_affine)
    if key in _BUILD_CACHE:
        return _BUILD_CACHE[key]

    nst = b_core // ST
    nc = bacc.Bacc("TRN2", debug=False, num_devices=N_CORES)

    # x.T fp8 packed: xt[st, p, kp, s, b] = SX*x.T[(2kp+s)*128 + p, st*ST + b]
    xt = nc.dram_tensor("xt", [nst, P, KP, 2, ST], F8, kind="ExternalInput").ap()
    yh = nc.dram_tensor("yh", [b_core, D], F16, kind="ExternalInput").ap()
    # W.T fp8 packed: wth[kp, p, s, o] = SW*W.T[(2kp+s)*128 + p, o]
    wth = nc.dram_tensor("wth", [KP, P, 2, D], F8, kind="ExternalInput").ap()
    # bias pair: [0] = SC*bias, [1] = 0; ones row for the K=1 bias matmul
    biash = nc.dram_tensor("biash", [1, 2, D], F8, kind="ExternalInput").ap()
    onesh = nc.dram_tensor("onesh", [1, 2, P], F8, kind="ExternalInput").ap()
    if not trivial_affine:
        gammah = nc.dram_tensor("gammah", [D], F32, kind="ExternalInput").ap()
        betah = nc.dram_tensor("betah", [D], F32, kind="ExternalInput").ap()
    out = nc.dram_tensor("out", [b_core, D], F16, kind="ExternalOutput").ap()

    with tile.TileContext(nc) as tc, ExitStack() as ctx:
        const = ctx.enter_context(tc.tile_pool(name="const", bufs=1))
        xtp = ctx.enter_context(tc.tile_pool(name="xtp", bufs=3))
        ypool = ctx.enter_context(tc.tile_pool(name="yp", bufs=6))
        work = ctx.enter_context(tc.tile_pool(name="work", bufs=12))
        stat = ctx.enter_context(tc.tile_pool(name="stat", bufs=12))
        psz = ctx.enter_context(tc.tile_pool(name="psz", bufs=8, space="PSUM"))

        # --- constants (ones/bias ride the sync ring ahead of the x loads:
        # they are tiny and gate the very first PE instruction) ---
        ones_sb = const.tile([1, 2, P], F8)
        nc.sync.dma_start(out=ones_sb[:], in_=onesh)
        bias_sb = const.tile([1, 2, D], F8)
        nc.sync.dma_start(out=bias_sb[:], in_=biash)
        wt_sb = const.tile([P, KP, 2, D], F8)  # [i_local, kp, s, o]
        for kp in range(KP):
            nc.scalar.dma_start(out=wt_sb[:, kp], in_=wth[kp])
        eps_sb = const.tile([P, 1], F32)
        nc.vector.memset(eps_sb[:], EPS * SC * SC)
        ones_f32 = const.tile([1, P], F32)
        nc.vector.memset(ones_f32[:], 1.0)
        warm_mov = const.tile([1, OB], F32)
        nc.vector.memset(warm_mov[:], 0.0)
        if not trivial_affine:
            gamma_f32 = const.tile([P, D], F32)
            nc.sync.dma_start(out=gamma_f32[:], in_=gammah.unsqueeze(0).to_broadcast([P, D]))
            gamma_sb = const.tile([P, D], F16)
            nc.scalar.activation(gamma_sb[:], gamma_f32[:], AF.Copy)
            beta_f32 = const.tile([P, D], F32)
            nc.sync.dma_start(out=beta_f32[:], in_=betah.unsqueeze(0).to_broadcast([P, D]))
            beta_sb = const.tile([P, D], F16)
            nc.scalar.activation(beta_sb[:], beta_f32[:], AF.Copy)

        # No PE warmup: this kernel is DVE-paced, and warmup matmuls only
        # delay tile 0's PSUM (which gates the whole DVE stream). The clock
        # ramp happens on the real matmuls either way.

        nb = b_core // P
        pair = {}
        for bt in range(nb):
            if bt % (ST // P) == 0:
                st = bt // (ST // P)
                xt_sb = xtp.tile([P, KP, 2, ST], F8)  # [i_local, kp, s, b]
                for kp in range(KP):
                    nc.sync.dma_start(out=xt_sb[:, kp], in_=xt[st, :, kp])
            j = bt % (ST // P)
            y_sb = ypool.tile([P, D], F16)
            nc.sync.dma_start(out=y_sb[:], in_=yh[bt * P:(bt + 1) * P, :])

            # --- matmuls: PSUM = SC*(x @ W.T + bias), fp8 DoubleRow.
            # Each column group gets its OWN PSUM tile so bn_stats (and the
            # normalize) of group g only waits for group g's accumulation
            # instead of the whole tile - the DVE stream starts earlier and
            # PSUM banks recycle at sub-tile granularity. Tile 0 uses 256-col
            # groups so the very first bn_stats fires ~2us sooner (it gates
            # the whole DVE stream, which is the kernel's critical path).
            ng = 2
            gw = D // ng
            z_hs = []
            stt = stat.tile([P, ng, 6], F32)
            for g in range(ng):
                z_ps = psz.tile([P, OB], F32)
                z_hs.append(z_ps)
                gs = bass.ts(g, gw)
                nc.tensor.matmul(
                    z_ps[:, 0:gw], ones_sb[:], bias_sb[:, :, gs],
                    start=True, stop=False, perf_mode=DR,
                )
                for kp in range(KP):
                    nc.tensor.matmul(
                        z_ps[:, 0:gw],
                        xt_sb[:, kp, :, bass.ts(j, P)],
                        wt_sb[:, kp, :, gs],
                        start=False, stop=(kp == KP - 1), perf_mode=DR,
                    )
                nc.vector.bn_stats(out=stt[:, g, :], in_=z_ps[:, 0:gw])

            # --- stats -> rstd in one ScalarE op: 1/sqrt(|var + eps|) ---
            mv = stat.tile([P, 2], F32)
            nc.vector.bn_aggr(out=mv[:], in_=stt[:])
            rstd = stat.tile([P, 1], F32)
            nc.scalar.activation(
                rstd[:], mv[:, 1:2], AF.Abs_reciprocal_sqrt,
                bias=eps_sb[:], scale=1.0,
            )
            nmr = stat.tile([P, 1], F32)  # -mean * rstd
            nc.vector.scalar_tensor_tensor(
                out=nmr[:], in0=mv[:, 0:1], scalar=-1.0, in1=rstd[:],
                op0=OP.mult, op1=OP.mult,
            )

            # --- ScalarE: t = (z - mean)*rstd per half (PSUM f32 -> fp16);
            # then u = t + y, o = u*y on DVE. The last two tiles run the
            # DVE chain per 512-col half to shorten the pipeline drain tail.
            rows = slice(bt * P, (bt + 1) * P)
            t_sb = work.tile([P, D], F16)
            u_sb = work.tile([P, D], F16)
            o_sb = work.tile([P, D], F16)
            for g in range(ng):
                gs = bass.ts(g, gw)
                nc.scalar.activation(
                    t_sb[:, gs], z_hs[g][:, 0:gw], AF.Identity,
                    bias=nmr[:], scale=rstd[:],
                )
                if not trivial_affine:
                    nc.vector.tensor_mul(out=t_sb[:, gs], in0=t_sb[:, gs], in1=gamma_sb[:, gs])
                    nc.vector.tensor_add(out=t_sb[:, gs], in0=t_sb[:, gs], in1=beta_sb[:, gs])
            chunks = 4 if bt >= nb - 2 else (2 if bt >= nb - 4 else 1)
            cw = D // chunks
            for q in range(chunks):
                cs = bass.ts(q, cw)
                nc.vector.tensor_add(out=u_sb[:, cs], in0=t_sb[:, cs], in1=y_sb[:, cs])
                nc.vector.tensor_mul(out=o_sb[:, cs], in0=u_sb[:, cs], in1=y_sb[:, cs])
                nc.sync.dma_start(out=out[rows, cs], in_=o_sb[:, cs])

    nc.finalize()
    _BUILD_CACHE[key] = nc
    return nc


def _run(nc, in_maps, **kwargs):
    return bass_utils.run_bass_kernel_spmd(
        nc, in_maps, core_ids=list(range(N_CORES)), **kwargs
    )


def _prepare(x, y, weight, bias, gamma, beta):
    x = np.asarray(x, dtype=np.float32)
    y = np.ascontiguousarray(y, dtype=np.float32)
    weight = np.asarray(weight, dtype=np.float32)
    bias = np.asarray(bias, dtype=np.float32)
    gamma = np.asarray(gamma, dtype=np.float32)
    beta = np.asarray(beta, dtype=np.float32)

    B, IN = x.shape
    assert IN == D and weight.shape == (D, D) and y.shape == (B, D)
    assert B % (N_CORES * ST) == 0
    b_core = B // N_CORES
    nst = b_core // ST

    trivial = bool(np.all(gamma == 1.0)) and bool(np.all(beta == 0.0))
    nc = _build(b_core, trivial)

    FP8 = ml_dtypes.float8_e4m3fn
    # W.T fp8 packed: [kp, p, s, o]
    wth_prep = np.ascontiguousarray(
        (weight.T * SW).astype(FP8).reshape(KP, 2, P, D).transpose(0, 2, 1, 3)
    )
    bias_prep = np.zeros((1, 2, D), dtype=FP8)
    bias_prep[0, 0, :] = (bias * SC).astype(FP8)
    ones_prep = np.ones((1, 2, P), dtype=FP8)

    in_maps = []
    for c in range(N_CORES):
        xs = (x[c * b_core:(c + 1) * b_core] * SX).astype(FP8)
        # x.T fp8 packed: [st, p, kp, s, b]
        xt_prep = np.ascontiguousarray(
            xs.T.reshape(KP, 2, P, nst, ST).transpose(3, 2, 0, 1, 4)
        )
        m = {
            "xt": xt_prep,
            "yh": np.ascontiguousarray(
                y[c * b_core:(c + 1) * b_core].astype(np.float16)
            ),
            "wth": wth_prep,
            "biash": bias_prep,
            "onesh": ones_prep,
        }
        if not trivial:
            m["gammah"] = gamma
            m["betah"] = beta
        in_maps.append(m)
    return nc, in_maps


def kernel(x, y, weight, bias, gamma, beta):
    nc, in_maps = _prepare(x, y, weight, bias, gamma, beta)
    res = _run(nc, in_maps)
    return np.concatenate(
        [r["out"].astype(np.float32) for r in res.results], axis=0
    )


# revision 39
# speedup vs baseline: 1.0096x; 1.0096x over previous
"""Fused Linear + LayerNorm + residual-multiply kernel for 8 Trainium2 cores.

Computes, for full inputs x[B,1024], y[B,1024], weight[1024,1024], bias, gamma, beta:
    z  = x @ weight.T + bias
    ln = (z - mean(z)) * rsqrt(var(z) + eps) * gamma + beta     (over last dim)
    out = (ln + y) * y
Data-parallel over the batch dim: each of the 8 NeuronCores processes B/8 rows.

v2: fp8 DoubleRow matmuls. Host casts x*SX and W.T*SW to fp8e4 (e4m3fn) and
packs them in k-pair layout; the PE runs DoubleRow matmuls (2 k-subtiles per
instruction, 0.5 cyc/row) so the z = x@W.T contraction streams in 1/4 the fp16
cycles. The scale c = SX*SW rides through PSUM and is undone by the LayerNorm
itself (mean/std scale together); only eps must be pre-scaled by c^2. Bias is
added in PSUM via a K=1 fp8 DoubleRow matmul from a ones row.

Per-core per 128-row tile (P=128, D=1024):
  - PE: bias matmul (start=True) + 4 k-pair x 2 half DoubleRow matmuls.
  - ScalarE: Copy PSUM f32 -> SBUF fp16 z_sb (so the DVE tail runs in 16-bit
    2x mode); later the Sqrt(var*1 + c^2 eps) activation.
  - VectorE: bn_stats/bn_aggr on z_sb for mean/var, reciprocal, -mean*rstd,
    then one fused AFFINE_THEN_ADD: u = (z_sb*rstd + nmr) + y  (= ln + y).
  - GpSimd: o = u * y (fp16), freeing the DVE for the next tile.
  - DMA: x/y loads on the sync HWDGE ring; W/bias once + out stores on the
    scalar ring (out alternates rings to balance sequencer time).
Output is fp16 in DRAM; the host upcasts to fp32. Total HBM traffic/core:
2MB x + 4MB y + 1MB W + 4MB out = 11MB.
"""

import numpy as np
import ml_dtypes
from contextlib import ExitStack

import concourse.bass as bass
import concourse.mybir as mybir
import concourse.tile as tile
from concourse import bacc, bass_utils

P = 128
D = 1024
KP = 4               # k-pairs (each pair = 2 k-subtiles of 128 = 256 contraction)
OB = 512             # o-block width (one PSUM bank of fp32)
ST = 512             # rows per x.T super-chunk
N_CORES = 8
EPS = 1e-5
SX = 4.0             # fp8 scale on x
SW = 64.0            # fp8 scale on W
SC = SX * SW         # PSUM carries SC * z

F32 = mybir.dt.float32
F16 = mybir.dt.float16
F8 = mybir.dt.float8e4

AF = mybir.ActivationFunctionType
OP = mybir.AluOpType
DR = mybir.MatmulPerfMode.DoubleRow

_BUILD_CACHE = {}


def _build(b_core: int, trivial_affine: bool):
    key = (b_core, trivial_affine)
    if key in _BUILD_CACHE:
        return _BUILD_CACHE[key]

    nst = b_core // ST
    nc = bacc.Bacc("TRN2", debug=False, num_devices=N_CORES)

    # x.T fp8 packed: xt[st, p, kp, s, b] = SX*x.T[(2kp+s)*128 + p, st*ST + b]
    xt = nc.dram_tensor("xt", [nst, P, KP, 2, ST], F8, kind="ExternalInput").ap()
    yh = nc.dram_tensor("yh", [b_core, D], F16, kind="ExternalInput").ap()
    # W.T fp8 packed: wth[kp, p, s, o] = SW*W.T[(2kp+s)*128 + p, o]
    wth = nc.dram_tensor("wth", [KP, P, 2, D], F8, kind="ExternalInput").ap()
    # bias pair: [0] = SC*bias, [1] = 0; ones row for the K=1 bias matmul
    biash = nc.dram_tensor("biash", [1, 2, D], F8, kind="ExternalInput").ap()
    onesh = nc.dram_tensor("onesh", [1, 2, P], F8, kind="ExternalInput").ap()
    if not trivial_affine:
        gammah = nc.dram_tensor("gammah", [D], F32, kind="ExternalInput").ap()
        betah = nc.dram_tensor("betah", [D], F32, kind="ExternalInput").ap()
    out = nc.dram_tensor("out", [b_core, D], F16, kind="ExternalOutput").ap()

    with tile.TileContext(nc) as tc, ExitStack() as ctx:
        const = ctx.enter_context(tc.tile_pool(name="const", bufs=1))
        xtp = ctx.enter_context(tc.tile_pool(name="xtp", bufs=3))
        ypool = ctx.enter_context(tc.tile_pool(name="yp", bufs=6))
        zpool = ctx.enter_context(tc.tile_pool(name="zp", bufs=4))
        upool = ctx.enter_context(tc.tile_pool(name="up", bufs=4))
        opool = ctx.enter_context(tc.tile_pool(name="op", bufs=4))
        stat = ctx.enter_context(tc.tile_pool(name="stat", bufs=12))
        psz = ctx.enter_context(tc.tile_pool(name="psz", bufs=8, space="PSUM"))

        # --- constants (ones/bias ride the sync ring ahead of the x loads:
        # they are tiny and gate the very first PE instruction) ---
        ones_sb = const.tile([1, 2, P], F8)
        nc.sync.dma_start(out=ones_sb[:], in_=onesh)
        bias_sb = const.tile([1, 2, D], F8)
        nc.sync.dma_start(out=bias_sb[:], in_=biash)
        wt_sb = const.tile([P, KP, 2, D], F8)  # [i_local, kp, s, o]
        for kp in range(KP):
            nc.scalar.dma_start(out=wt_sb[:, kp], in_=wth[kp])
        eps_sb = const.tile([P, 1], F32)
        nc.vector.memset(eps_sb[:], EPS * SC * SC)
        ones_f32 = const.tile([1, P], F32)
        nc.vector.memset(ones_f32[:], 1.0)
        warm_mov = const.tile([1, OB], F32)
        nc.vector.memset(warm_mov[:], 0.0)
        if not trivial_affine:
            gamma_f32 = const.tile([P, D], F32)
            nc.sync.dma_start(out=gamma_f32[:], in_=gammah.unsqueeze(0).to_broadcast([P, D]))
            gamma_sb = const.tile([P, D], F16)
            nc.scalar.activation(gamma_sb[:], gamma_f32[:], AF.Copy)
            beta_f32 = const.tile([P, D], F32)
            nc.sync.dma_start(out=beta_f32[:], in_=betah.unsqueeze(0).to_broadcast([P, D]))
            beta_sb = const.tile([P, D], F16)
            nc.scalar.activation(beta_sb[:], beta_f32[:], AF.Copy)

        # No PE warmup: this kernel is DVE-paced, and warmup matmuls only
        # delay tile 0's PSUM (which gates the whole DVE stream). The clock
        # ramp happens on the real matmuls either way.

        nb = b_core // P
        pair = {}
        for bt in range(nb):
            if bt % (ST // P) == 0:
                st = bt // (ST // P)
                xt_sb = xtp.tile([P, KP, 2, ST], F8)  # [i_local, kp, s, b]
                for kp in range(KP):
                    nc.sync.dma_start(out=xt_sb[:, kp], in_=xt[st, :, kp])
            j = bt % (ST // P)
            y_sb = ypool.tile([P, D], F16)
            nc.sync.dma_start(out=y_sb[:], in_=yh[bt * P:(bt + 1) * P, :])

            # --- matmuls: PSUM = SC*(x @ W.T + bias), fp8 DoubleRow.
            # Each column group gets its OWN PSUM tile so bn_stats (and the
            # normalize) of group g only waits for group g's accumulation
            # instead of the whole tile - the DVE stream starts earlier and
            # PSUM banks recycle at sub-tile granularity. Tile 0 uses 256-col
            # groups so the very first bn_stats fires ~2us sooner (it gates
            # the whole DVE stream, which is the kernel's critical path).
            ng = 2
            gw = D // ng
            z_hs = []
            stt = stat.tile([P, ng, 6], F32)
            for g in range(ng):
                z_ps = psz.tile([P, OB], F32)
                z_hs.append(z_ps)
                gs = bass.ts(g, gw)
                nc.tensor.matmul(
                    z_ps[:, 0:gw], ones_sb[:], bias_sb[:, :, gs],
                    start=True, stop=False, perf_mode=DR,
                )
                for kp in range(KP):
                    nc.tensor.matmul(
                        z_ps[:, 0:gw],
                        xt_sb[:, kp, :, bass.ts(j, P)],
                        wt_sb[:, kp, :, gs],
                        start=False, stop=(kp == KP - 1), perf_mode=DR,
                    )
                nc.vector.bn_stats(out=stt[:, g, :], in_=z_ps[:, 0:gw])

            # --- stats -> rstd in one ScalarE op: 1/sqrt(|var + eps|) ---
            mv = stat.tile([P, 2], F32)
            nc.vector.bn_aggr(out=mv[:], in_=stt[:])
            rstd = stat.tile([P, 1], F32)
            nc.scalar.activation(
                rstd[:], mv[:, 1:2], AF.Abs_reciprocal_sqrt,
                bias=eps_sb[:], scale=1.0,
            )
            nmr = stat.tile([P, 1], F32)  # -mean * rstd
            nc.vector.scalar_tensor_tensor(
                out=nmr[:], in0=mv[:, 0:1], scalar=-1.0, in1=rstd[:],
                op0=OP.mult, op1=OP.mult,
            )

            # --- ScalarE: t = (z - mean)*rstd per half (PSUM f32 -> fp16);
            # then u = t + y, o = u*y on DVE. The last two tiles run the
            # DVE chain per 512-col half to shorten the pipeline drain tail.
            rows = slice(bt * P, (bt + 1) * P)
            t_sb = zpool.tile([P, D], F16)
            u_sb = upool.tile([P, D], F16)
            o_sb = opool.tile([P, D], F16)
            for g in range(ng):
                gs = bass.ts(g, gw)
                nc.scalar.activation(
                    t_sb[:, gs], z_hs[g][:, 0:gw], AF.Identity,
                    bias=nmr[:], scale=rstd[:],
                )
                if not trivial_affine:
                    nc.vector.tensor_mul(out=t_sb[:, gs], in0=t_sb[:, gs], in1=gamma_sb[:, gs])
                    nc.vector.tensor_add(out=t_sb[:, gs], in0=t_sb[:, gs], in1=beta_sb[:, gs])
            chunks = 4 if bt == nb - 1 else (2 if bt >= nb - 3 else 1)
            cw = D // chunks
            for q in range(chunks):
                cs = bass.ts(q, cw)
                nc.vector.tensor_add(out=u_sb[:, cs], in0=t_sb[:, cs], in1=y_sb[:, cs])
                nc.vector.tensor_mul(out=o_sb[:, cs], in0=u_sb[:, cs], in1=y_sb[:, cs])
                nc.sync.dma_start(out=out[rows, cs], in_=o_sb[:, cs])

    nc.finalize()
    _BUILD_CACHE[key] = nc
    return nc


def _run(nc, in_maps, **kwargs):
    return bass_utils.run_bass_kernel_spmd(
        nc, in_maps, core_ids=list(range(N_CORES)), **kwargs
    )


def _prepare(x, y, weight, bias, gamma, beta):
    x = np.asarray(x, dtype=np.float32)
    y = np.ascontiguousarray(y, dtype=np.float32)
    weight = np.asarray(weight, dtype=np.float32)
    bias = np.asarray(bias, dtype=np.float32)
    gamma = np.asarray(gamma, dtype=np.float32)
    beta = np.asarray(beta, dtype=np.float32)

    B, IN = x.shape
    assert IN == D and weight.shape == (D, D) and y.shape == (B, D)
    assert B % (N_CORES * ST) == 0
    b_core = B // N_CORES
    nst = b_core // ST

    trivial = bool(np.all(gamma == 1.0)) and bool(np.all(beta == 0.0))
    nc = _build(b_core, trivial)

    FP8 = ml_dtypes.float8_e4m3fn
    # W.T fp8 packed: [kp, p, s, o]
    wth_prep = np.ascontiguousarray(
        (weight.T * SW).astype(FP8).reshape(KP, 2, P, D).transpose(0, 2, 1, 3)
    )
    bias_prep = np.zeros((1, 2, D), dtype=FP8)
    bias_prep[0, 0, :] = (bias * SC).astype(FP8)
    ones_prep = np.ones((1, 2, P), dtype=FP8)

    in_maps = []
    for c in range(N_CORES):
        xs = (x[c * b_core:(c + 1) * b_core] * SX).astype(FP8)
        # x.T fp8 packed: [st, p, kp, s, b]
        xt_prep = np.ascontiguousarray(
            xs.T.reshape(KP, 2, P, nst, ST).transpose(3, 2, 0, 1, 4)
        )
        m = {
            "xt": xt_prep,
            "yh": np.ascontiguousarray(
                y[c * b_core:(c + 1) * b_core].astype(np.float16)
            ),
            "wth": wth_prep,
            "biash": bias_prep,
            "onesh": ones_prep,
        }
        if not trivial:
            m["gammah"] = gamma
            m["betah"] = beta
        in_maps.append(m)
    return nc, in_maps


def kernel(x, y, weight, bias, gamma, beta):
    nc, in_maps = _prepare(x, y, weight, bias, gamma, beta)
    res = _run(nc, in_maps)
    return np.concatenate(
        [r["out"].astype(np.float32) for r in res.results], axis=0
    )
